# revision 2
# baseline (speedup 1.0000x reference)
"""Multi-head latent attention kernel for Trainium2 (8 NeuronCores).

Strategy: data-parallel over batch (B=8 -> 1 batch element per core, no
collectives). Per core, two phases:

  Phase 1 (projection): stream x.T in s-blocks; compute k.T (o-major) with
  fp32r matmuls, add bias + RoPE (pair-rotation via a small constant matmul),
  spill RoPE'd k.T to HBM scratch in fp16; compute v (s-major) and spill fp16.

  Phase 2 (attention): latent queries projected + RoPE'd once (fp32r); per
  head-pair: scores = lq_rot @ k_rot.T (fp16 matmuls, two heads packed in the
  128-row PE array), softmax with running per-chunk max + exp on the scalar
  engine (row-sum via activation accum_out), attn normalized, transposed via
  the PE, attn.T @ v accumulated into a.T, then the output projection (fp16)
  with v-bias and out-bias folded into a host-precomputed effective bias
  (softmax rows sum to 1, so v's bias contributes exactly v_b @ out_w.T).

All big matmuls use float32r (TF32-like, 1 cycle/row) or fp16 (1 cycle/row).
"""

import numpy as np

import concourse.mybir as mybir
import concourse.tile as tile
from concourse import bacc
from concourse.bass_utils import run_bass_kernel_spmd

# Problem shape (hardcoded per contract)
B, S, D, L, H = 8, 4096, 1024, 64, 16
DH = D // H          # 64
DC = D // 128        # 8 contraction chunks
OC = D // 128        # 8 output chunks
NSB = 8              # s-blocks
SBLK = S // NSB      # 512
NSC = S // 128       # 32 s-chunks of 128
F32R = mybir.dt.float32r
F32 = mybir.dt.float32
F16 = mybir.dt.float16


def build_nc(reps=1):
    nc = bacc.Bacc("TRN2", target_bir_lowering=False, debug=False, num_devices=8)
    dp = lambda n, s, d: nc.dram_tensor(n, s, d, kind="ExternalInput").ap()

    xT = dp("xT", [D, S], F32R)
    kwT = dp("kwT", [D, D], F32R)
    vwT = dp("vwT", [D, D], F32R)
    qwT = dp("qwT", [D, D], F32R)
    latT = dp("latT", [D, L], F32R)
    owT = dp("owT", [D, D], F16)
    rT = dp("rT", [128, 128], F16)
    ident = dp("ident", [128, 128], F16)
    cosS = dp("cosS", [128, S], F16)
    sinS = dp("sinS", [128, S], F16)
    cosL = dp("cosL", [128, L], F16)
    sinL = dp("sinL", [128, L], F16)
    kbT = dp("kbT", [128, OC], F32)
    qbT = dp("qbT", [128, OC], F32)
    outb = dp("outb", [L, D], F32)

    out = nc.dram_tensor("out", [L, D], F32, kind="ExternalOutput").ap()
    kS = nc.dram_tensor("kS", [OC, 128, S], F16).ap()
    vS = nc.dram_tensor("vS", [NSC, 128, D], F16).ap()

    xT_v = xT.rearrange("(dc p) (sb j) -> p dc sb j", p=128, sb=NSB)
    kwT_v = kwT.rearrange("(dc p) (oc m) -> p dc oc m", p=128, oc=OC)
    vwT_v = vwT.rearrange("(dc p) o -> p dc o", p=128)
    qwT_v = qwT.rearrange("(dc p) (oc m) -> p dc oc m", p=128, oc=OC)
    latT_v = latT.rearrange("(dc p) l -> p dc l", p=128)
    owT_v = owT.rearrange("(ic p) (jc m) -> p ic jc m", p=128, jc=2)

    AX = mybir.AxisListType.X
    ACT_ID = mybir.ActivationFunctionType.Identity
    ACT_EXP = mybir.ActivationFunctionType.Exp

    with tile.TileContext(nc) as tc:
        with tc.tile_pool(name="const", bufs=1) as cpool:
            rT_t = cpool.tile([128, 128], F16)
            nc.sync.dma_start(out=rT_t[:], in_=rT[:])
            id_t = cpool.tile([128, 128], F16)
            nc.sync.dma_start(out=id_t[:], in_=ident[:])
            cosL_t = cpool.tile([128, L], F16)
            nc.sync.dma_start(out=cosL_t[:], in_=cosL[:])
            sinL_t = cpool.tile([128, L], F16)
            nc.sync.dma_start(out=sinL_t[:], in_=sinL[:])
            kbT_t = cpool.tile([128, OC], F32)
            nc.sync.dma_start(out=kbT_t[:], in_=kbT[:])
            qbT_t = cpool.tile([128, OC], F32)
            nc.sync.dma_start(out=qbT_t[:], in_=qbT[:])
            outb_t = cpool.tile([L, D], F32)
            nc.sync.dma_start(out=outb_t[:], in_=outb[:])
            owT_t = cpool.tile([128, OC, 2, 512], F16)
            nc.sync.dma_start(out=owT_t[:], in_=owT_v)
            latT_t = cpool.tile([128, DC, L], F32R)
            nc.sync.dma_start(out=latT_t[:], in_=latT_v)
            lqr_t = cpool.tile([128, OC, L], F16)
            aT_t = cpool.tile([128, OC, L], F16)

            for rep in range(reps):
                # ---- latent query projection + RoPE (small) ----
                with tc.tile_pool(name="qw", bufs=1) as qwp, \
                     tc.tile_pool(name="lqtmp", bufs=4) as lqtmp, \
                     tc.tile_pool(name="psq", bufs=4, space="PSUM") as psqp:
                    qwT_t = qwp.tile([128, DC, OC, 128], F32R)
                    nc.sync.dma_start(out=qwT_t[:], in_=qwT_v)
                    for oc in range(OC):
                        psq = psqp.tile([128, L], F32)
                        for dc in range(DC):
                            nc.tensor.matmul(
                                psq[:], qwT_t[:, dc, oc, :], latT_t[:, dc, :],
                                start=(dc == 0), stop=(dc == DC - 1))
                        lqb = lqtmp.tile([128, L], F16, name="lqb")
                        nc.scalar.activation(lqb[:], psq[:], ACT_ID,
                                             bias=qbT_t[:, oc:oc + 1])
                        psr = psqp.tile([128, L], F32, name="psr")
                        nc.tensor.matmul(psr[:], rT_t[:], lqb[:],
                                         start=True, stop=True)
                        t1 = lqtmp.tile([128, L], F16, name="t1")
                        nc.vector.tensor_mul(t1[:], lqb[:], cosL_t[:])
                        u1 = lqtmp.tile([128, L], F16, name="u1")
                        nc.vector.tensor_mul(u1[:], psr[:], sinL_t[:])
                        nc.vector.tensor_add(lqr_t[:, oc, :], t1[:], u1[:])

                # ---- phase 1: k/v projections, RoPE k, spill fp16 ----
                with tc.tile_pool(name="kvw", bufs=1) as kvwp, \
                     tc.tile_pool(name="xblk", bufs=2) as xbp, \
                     tc.tile_pool(name="tab", bufs=2) as tabp, \
                     tc.tile_pool(name="p1tmp", bufs=3) as p1tmp, \
                     tc.tile_pool(name="psk", bufs=2, space="PSUM") as pskp, \
                     tc.tile_pool(name="psr1", bufs=2, space="PSUM") as psrp, \
                     tc.tile_pool(name="psv", bufs=2, space="PSUM") as psvp:
                    kwT_t = kvwp.tile([128, DC, OC, 128], F32R)
                    nc.sync.dma_start(out=kwT_t[:], in_=kwT_v)
                    vwT_t = kvwp.tile([128, DC, D], F32R)
                    nc.sync.dma_start(out=vwT_t[:], in_=vwT_v)
                    for sb in range(NSB):
                        xblk = xbp.tile([128, DC, SBLK], F32R)
                        nc.sync.dma_start(out=xblk[:], in_=xT_v[:, :, sb, :])
                        cosb = tabp.tile([128, SBLK], F16, name="cosb")
                        nc.sync.dma_start(
                            out=cosb[:], in_=cosS[:, sb * SBLK:(sb + 1) * SBLK])
                        sinb = tabp.tile([128, SBLK], F16, name="sinb")
                        nc.sync.dma_start(
                            out=sinb[:], in_=sinS[:, sb * SBLK:(sb + 1) * SBLK])
                        for oc in range(OC):
                            psk = pskp.tile([128, SBLK], F32)
                            for dc in range(DC):
                                nc.tensor.matmul(
                                    psk[:], kwT_t[:, dc, oc, :], xblk[:, dc, :],
                                    start=(dc == 0), stop=(dc == DC - 1))
                            kb = p1tmp.tile([128, SBLK], F16, name="kb")
                            nc.scalar.activation(kb[:], psk[:], ACT_ID,
                                                 bias=kbT_t[:, oc:oc + 1])
                            psr1 = psrp.tile([128, SBLK], F32)
                            nc.tensor.matmul(psr1[:], rT_t[:], kb[:],
                                             start=True, stop=True)
                            t1 = p1tmp.tile([128, SBLK], F16, name="t1")
                            nc.vector.tensor_mul(t1[:], kb[:], cosb[:])
                            u1 = p1tmp.tile([128, SBLK], F16, name="u1")
                            nc.vector.tensor_mul(u1[:], psr1[:], sinb[:])
                            kr = p1tmp.tile([128, SBLK], F16, name="kr")
                            nc.vector.tensor_add(kr[:], t1[:], u1[:])
                            nc.sync.dma_start(
                                out=kS[oc, :, sb * SBLK:(sb + 1) * SBLK],
                                in_=kr[:])
                        for ss in range(4):
                            s0 = ss * 128
                            vtile = p1tmp.tile([128, D], F16, name="vt")
                            for jc in range(2):
                                psv = psvp.tile([128, 512], F32)
                                for dc in range(DC):
                                    nc.tensor.matmul(
                                        psv[:], xblk[:, dc, s0:s0 + 128],
                                        vwT_t[:, dc, jc * 512:(jc + 1) * 512],
                                        start=(dc == 0), stop=(dc == DC - 1))
                                nc.scalar.copy(
                                    vtile[:, jc * 512:(jc + 1) * 512], psv[:])
                            nc.sync.dma_start(out=vS[sb * 4 + ss, :, :],
                                              in_=vtile[:])

                # ---- phase 2: attention per head-pair ----
                with tc.tile_pool(name="krh", bufs=2) as krhp, \
                     tc.tile_pool(name="vh", bufs=2) as vhp, \
                     tc.tile_pool(name="scb", bufs=2) as scbp, \
                     tc.tile_pool(name="attn", bufs=2) as attnp, \
                     tc.tile_pool(name="smax", bufs=4) as smaxp, \
                     tc.tile_pool(name="att", bufs=4) as attp, \
                     tc.tile_pool(name="pss", bufs=2, space="PSUM") as pssp, \
                     tc.tile_pool(name="pst", bufs=2, space="PSUM") as pstp, \
                     tc.tile_pool(name="psav", bufs=2, space="PSUM") as psavp:
                    for hp in range(OC):
                        krh = krhp.tile([128, S], F16)
                        nc.sync.dma_start(out=krh[:], in_=kS[hp])
                        vh = vhp.tile([128, NSC, 128], F16)
                        nc.sync.dma_start(
                            out=vh[:],
                            in_=vS[:, :, hp * 128:(hp + 1) * 128].rearrange(
                                "sc p j -> p sc j"))
                        scb = scbp.tile([128, NSB, SBLK], F16)
                        m8 = smaxp.tile([128, NSB], F32, name="m8")
                        for sc in range(NSB):
                            pss = pssp.tile([128, SBLK], F32)
                            j0 = sc * SBLK
                            nc.tensor.matmul(
                                pss[0:64, :], lqr_t[0:64, hp, :],
                                krh[0:64, j0:j0 + SBLK],
                                start=True, stop=True, tile_position=(0, 0))
                            nc.tensor.matmul(
                                pss[64:128, :], lqr_t[64:128, hp, :],
                                krh[64:128, j0:j0 + SBLK],
                                start=True, stop=True, tile_position=(64, 64))
                            nc.scalar.copy(scb[:, sc, :], pss[:])
                            nc.vector.reduce_max(
                                out=m8[:, sc:sc + 1], in_=pss[:], axis=AX)
                        m = smaxp.tile([128, 1], F32, name="m")
                        nc.vector.reduce_max(out=m[:], in_=m8[:], axis=AX)
                        negm = smaxp.tile([128, 1], F32, name="negm")
                        nc.vector.tensor_scalar_mul(negm[:], m[:], -1.0)
                        attn = attnp.tile([128, NSB, SBLK], F16)
                        z8 = smaxp.tile([128, NSB], F32, name="z8")
                        for sc in range(NSB):
                            nc.scalar.activation(
                                attn[:, sc, :], scb[:, sc, :], ACT_EXP,
                                bias=negm[:], scale=1.0,
                                accum_out=z8[:, sc:sc + 1])
                        z = smaxp.tile([128, 1], F32, name="z")
                        nc.vector.reduce_sum(out=z[:], in_=z8[:], axis=AX)
                        rz = smaxp.tile([128, 1], F32, name="rz")
                        nc.vector.reciprocal(rz[:], z[:])
                        attn_f = attn.rearrange("p a b -> p (a b)")
                        nc.vector.tensor_scalar_mul(attn_f, attn_f, rz[:])
                        psav = psavp.tile([128, L], F32)
                        for ch in range(NSC):
                            pst = pstp.tile([128, 128], F16)
                            nc.tensor.transpose(
                                pst[:], attn_f[:, ch * 128:(ch + 1) * 128],
                                id_t[:])
                            att = attp.tile([128, 128], F16)
                            if ch % 2 == 0:
                                nc.scalar.copy(att[:], pst[:])
                            else:
                                nc.vector.tensor_copy(att[:], pst[:])
                            nc.tensor.matmul(
                                psav[0:64, :], vh[:, ch, 0:64], att[:, 0:64],
                                start=(ch == 0), stop=(ch == NSC - 1),
                                tile_position=(0, 0), skip_group_check=True)
                            nc.tensor.matmul(
                                psav[64:128, :], vh[:, ch, 64:128],
                                att[:, 64:128],
                                start=(ch == 0), stop=(ch == NSC - 1),
                                tile_position=(0, 64), skip_group_check=True)
                        nc.scalar.copy(aT_t[:, hp, :], psav[:])

                # ---- output projection ----
                with tc.tile_pool(name="fin", bufs=2) as finp, \
                     tc.tile_pool(name="pso", bufs=2, space="PSUM") as psop:
                    for jc in range(2):
                        pso = psop.tile([L, 512], F32)
                        for ic in range(OC):
                            nc.tensor.matmul(
                                pso[:], aT_t[:, ic, :], owT_t[:, ic, jc, :],
                                start=(ic == 0), stop=(ic == OC - 1))
                        fin = finp.tile([L, 512], F32)
                        nc.vector.tensor_add(
                            fin[:], pso[:], outb_t[:, jc * 512:(jc + 1) * 512])
                        nc.sync.dma_start(
                            out=out[:, jc * 512:(jc + 1) * 512], in_=fin[:])
    nc.compile()
    return nc


def host_prep(x, latents, q_w, q_b, k_w, k_b, v_w, v_b, out_w, out_b):
    """Build per-core input maps (all host-side numpy)."""
    f32 = np.float32
    scale = 1.0 / np.sqrt(np.float32(DH))
    kwT = np.ascontiguousarray(k_w.T).astype(f32)
    vwT = np.ascontiguousarray(v_w.T).astype(f32)
    qwT = np.ascontiguousarray((q_w * scale).T).astype(f32)
    owT = np.ascontiguousarray(out_w.T).astype(np.float16)
    latT = np.ascontiguousarray(latents.T).astype(f32)
    kbT = np.ascontiguousarray(k_b.reshape(OC, 128).T).astype(f32)
    qbT = np.ascontiguousarray((q_b * scale).reshape(OC, 128).T).astype(f32)
    outb = np.ascontiguousarray(
        np.broadcast_to(out_b + v_b @ out_w.T, (L, D))).astype(f32)

    rTm = np.zeros((128, 128), np.float16)
    for p in range(128):
        if p % 2 == 0:
            rTm[p + 1, p] = -1.0     # rot[p] = -k[p+1] for even p
        else:
            rTm[p - 1, p] = 1.0      # rot[p] = +k[p-1] for odd p
    ident = np.eye(128, dtype=np.float16)

    inv = (1.0 / (10000.0 ** (np.arange(0, DH, 2, dtype=f32) / DH)))  # [32]
    pair_idx = (np.arange(128) % DH) // 2                             # [128]
    angS = np.arange(S, dtype=f32)[:, None] * inv[None, :]            # [S, 32]
    cosS = np.cos(angS)[:, pair_idx].T.astype(np.float16)             # [128, S]
    sinS = np.sin(angS)[:, pair_idx].T.astype(np.float16)
    angL = np.arange(L, dtype=f32)[:, None] * inv[None, :]
    cosL = np.cos(angL)[:, pair_idx].T.astype(np.float16)
    sinL = np.sin(angL)[:, pair_idx].T.astype(np.float16)

    shared = dict(kwT=kwT, vwT=vwT, qwT=qwT, owT=owT, latT=latT, kbT=kbT,
                  qbT=qbT, outb=outb, rT=rTm, ident=ident, cosS=cosS,
                  sinS=sinS, cosL=cosL, sinL=sinL)
    in_maps = []
    for c in range(B):
        m = dict(shared)
        m["xT"] = np.ascontiguousarray(x[c].T).astype(f32)
        in_maps.append(m)
    return in_maps


_NC_CACHE = {}


def get_nc(reps=1):
    if reps not in _NC_CACHE:
        _NC_CACHE[reps] = build_nc(reps)
    return _NC_CACHE[reps]


def kernel(**inputs):
    np_inputs = {k: np.asarray(v) for k, v in inputs.items()}
    in_maps = host_prep(**np_inputs)
    nc = get_nc(1)
    res = run_bass_kernel_spmd(nc, in_maps, list(range(B)))
    return np.stack([res.results[c]["out"] for c in range(B)], axis=0)


if __name__ == "__main__":
    rng = np.random.default_rng(0)
    ins = {
        "x": rng.standard_normal((B, S, D)).astype(np.float32),
        "latents": rng.standard_normal((L, D)).astype(np.float32),
    }
    for n in ["q", "k", "v", "out"]:
        ins[f"{n}_w"] = (rng.standard_normal((D, D)) * 0.02).astype(np.float32)
        ins[f"{n}_b"] = (rng.standard_normal((D,)) * 0.02).astype(np.float32)
    out = kernel(**ins)
    print("out", out.shape, out.dtype, float(np.abs(out).mean()))
